# revision 1
# baseline (speedup 1.0000x reference)
import sys
for _p in ("/opt/trn_rl_repo",):
    if _p not in sys.path:
        sys.path.insert(0, _p)
"""GAT 2-layer kernel for TRN2, 8-core dst-sharded, window/tile streaming.

Algorithm (matches reference.py math exactly, segment-max skipped since
softmax is shift-invariant and scores are small):
  layer0: h = x@W0 [N,256]; al_s/al_d = x@w_as0/w_ad0 [N,8]
          per edge e=(s,d): ex = exp(leakyrelu(al_s[s]+al_d[d], 0.2))
          U[d] = sum_e ex * h[s];  den[d] = sum_e ex
          gat0 = U/(den+1e-16); h1in = ELU(gat0 + x@Wr0 + b0 + br0)
  layer1: same with W1/a1 on h1in, out = gat1 + b1 + h1in@Wr1 + br1
          out = log_softmax(out, axis=1)

Sharding: core c owns dst nodes [c*OWN,(c+1)*OWN); edges bucketed by
dst-window (128 nodes); gather tables (h|al_src packed rows) are
AllGathered; per-window dma_gather with int16 idx via lo/hi base split.
"""

import numpy as np
from contextlib import ExitStack

import concourse.bass as bass
import concourse.bacc as bacc
import concourse.mybir as mybir
import concourse.tile as tile

dt = mybir.dt
F32 = dt.float32
AL = mybir.AluOpType
ACT = mybir.ActivationFunctionType

NEG_SLOPE = 0.2


# ----------------------------------------------------------------- host prep

def make_cfg(N, E, F_in, H, D, OUT, cores, split=32000):
    assert N % cores == 0
    own = N // cores
    cfg = dict(
        N=N, E=E, F_IN=F_in, H=H, D=D, OUT=OUT, CORES=cores, OWN=own,
        WIN=128,
        NW=(own + 127) // 128,
        SPLIT=min(split, N),          # lo src range [0, SPLIT)
        ROW0=(F_in + H + 63) // 64 * 64,   # table0 row f32 elems (256B mult)
        ROW1=(OUT + 1 + 63) // 64 * 64,    # table1 row f32 elems
    )
    # region rows: [S][0..N-1][S2]
    cfg["NROWS"] = N + 2
    cfg["SENT_A"] = 0                    # idxA of sentinel S (region row 0)
    cfg["SENT_B"] = N + 1 - (cfg["SPLIT"] + 1)  # idxB of S2
    assert cfg["SPLIT"] + 1 <= 32767 and cfg["SENT_B"] <= 32767
    return cfg


def wrap_idx(iv):
    """[n*16] int -> [128, n] int16 gather layout (t -> (t%16, t//16))."""
    iv = np.asarray(iv, np.int64)
    assert len(iv) % 16 == 0
    cols = len(iv) // 16
    a = iv.reshape(cols, 16).T.astype(np.int16)
    return np.tile(a, (8, 1))


def prepare(x, edge_index, weights, cfg):
    """Host-side sharding. Returns (in_maps, meta). weights: dict of np arrays."""
    N, E, CORES, OWN, WIN, NW = (cfg[k] for k in
                                 ("N", "E", "CORES", "OWN", "WIN", "NW"))
    SPLIT = cfg["SPLIT"]
    H, D, F_IN, OUT = cfg["H"], cfg["D"], cfg["F_IN"], cfg["OUT"]

    src = np.concatenate([np.asarray(edge_index[0]), np.arange(N)]).astype(np.int64)
    dst = np.concatenate([np.asarray(edge_index[1]), np.arange(N)]).astype(np.int64)

    owner = dst // OWN
    winid = (dst - owner * OWN) // WIN          # window within core
    hi = (src >= SPLIT).astype(np.int64)

    # counts[c, w, lohi]
    counts = np.zeros((CORES, NW, 2), np.int64)
    np.add.at(counts, (owner, winid, hi), 1)
    lo_slots = np.maximum(128, np.ceil(counts[:, :, 0].max(0) / 128).astype(np.int64) * 128)
    hi_slots = np.maximum(128, np.ceil(counts[:, :, 1].max(0) / 128).astype(np.int64) * 128)
    tpw = (lo_slots + hi_slots) // 128          # tiles per window [NW]
    t_tiles = int(tpw.sum())
    idx_cols = int((lo_slots + hi_slots).sum() // 16)

    # per-core edge arrays
    in_maps = []
    order = np.lexsort((hi, winid, owner))      # group by (owner, window, lohi)
    src_s, dst_s, own_s, win_s, hi_s = (a[order] for a in (src, dst, owner, winid, hi))
    # boundaries per (core, window, lohi)
    cnt_sorted = counts  # [c,w,2]
    for c in range(CORES):
        idx_list = []
        dstc = np.full((128, t_tiles), -1.0, np.float32)
        tile_base = 0
        # slice this core's edges
        c_mask_start = np.searchsorted(own_s, c)
        c_mask_end = np.searchsorted(own_s, c + 1)
        cs, cd, cw, ch = (a[c_mask_start:c_mask_end]
                          for a in (src_s, dst_s, win_s, hi_s))
        pos = 0
        for w in range(NW):
            n_lo = int(cnt_sorted[c, w, 0]); n_hi = int(cnt_sorted[c, w, 1])
            L = int(lo_slots[w]); Hs = int(hi_slots[w])
            s_lo = cs[pos:pos + n_lo]; d_lo = cd[pos:pos + n_lo]
            s_hi = cs[pos + n_lo:pos + n_lo + n_hi]
            d_hi = cd[pos + n_lo:pos + n_lo + n_hi]
            pos += n_lo + n_hi
            idxA = np.full(L, cfg["SENT_A"], np.int64)
            idxA[:n_lo] = s_lo + 1
            idxB = np.full(Hs, cfg["SENT_B"], np.int64)
            idxB[:n_hi] = s_hi - SPLIT
            idx_list.append(wrap_idx(idxA))
            idx_list.append(wrap_idx(idxB))
            dl = np.full(L + Hs, -1.0, np.float32)
            dl[:n_lo] = (d_lo - c * OWN - w * WIN).astype(np.float32)
            dl[L:L + n_hi] = (d_hi - c * OWN - w * WIN).astype(np.float32)
            nt = (L + Hs) // 128
            dstc[:, tile_base:tile_base + nt] = dl.reshape(nt, 128).T
            tile_base += nt
        idx_arr = np.concatenate(idx_list, axis=1)
        assert idx_arr.shape == (128, idx_cols)
        xT = np.ascontiguousarray(x[c * OWN:(c + 1) * OWN].T)  # [F_IN, OWN]
        in_maps.append(dict(xT=xT, idx=idx_arr, dstc=dstc))

    # constants (shared across cores)
    W0 = weights["W0"]; a_s0 = weights["a_src0"]; a_d0 = weights["a_dst0"]
    Wr0 = weights["Wr0"]; W1 = weights["W1"]; a_s1 = weights["a_src1"]
    a_d1 = weights["a_dst1"]; Wr1 = weights["Wr1"]
    blk0s = np.zeros((H * D, H), np.float32)
    blk0d = np.zeros((H * D, H), np.float32)
    for k in range(H):
        blk0s[k * D:(k + 1) * D, k] = a_s0[k]
        blk0d[k * D:(k + 1) * D, k] = a_d0[k]
    w_as0 = (W0 @ blk0s).astype(np.float32)     # [F_IN, H]
    w_ad0 = (W0 @ blk0d).astype(np.float32)
    wcat0 = np.concatenate([W0, w_as0, w_ad0, Wr0], axis=1).astype(np.float32)
    # [F_IN, F_IN + 2H + F_IN] = e.g. [256, 528]
    w_as1 = (W1 @ a_s1.reshape(-1, 1)).astype(np.float32)   # [HD, 1]
    w_ad1 = (W1 @ a_d1.reshape(-1, 1)).astype(np.float32)
    wcat1 = np.concatenate([W1, w_as1, w_ad1, Wr1], axis=1).astype(np.float32)
    # [HD, OUT+2+OUT] = [256, 130]
    bias0 = (weights["b0"] + weights["br0"]).astype(np.float32)  # [256]
    bias1 = (weights["b1"] + weights["br1"]).astype(np.float32)  # [64]
    bias_row1 = np.zeros((1, wcat1.shape[1]), np.float32)
    bias_row1[0, OUT + 2:] = bias1
    consts = dict(
        wcat0=wcat0,
        bias0=np.ascontiguousarray(bias0.reshape(2, 128).T),  # [128,2] chunks
        wcat1=wcat1, bias_row1=bias_row1,
        ident=np.eye(128, dtype=np.float32),
        iota=np.tile(np.arange(128, dtype=np.float32), (128, 1)),
        ones_row=np.ones((1, 128), np.float32),
    )
    for m in in_maps:
        m.update(consts)
    meta = dict(lo_slots=lo_slots.tolist(), hi_slots=hi_slots.tolist(),
                tpw=tpw.tolist(), t_tiles=t_tiles, idx_cols=idx_cols)
    return in_maps, meta


# --------------------------------------------------------------- bass builder

def build(cfg, meta, gdt=F32, mmdt=F32, repeat=1, sub=7, gbufs=3, vbufs=3, wbufs=3):
    """Build the Bacc program. gdt: gather-table dtype knob (f32 now).
    mmdt: matmul operand dtype for the aggregation (f32 / float32r)."""
    N, CORES, OWN, WIN, NW = (cfg[k] for k in ("N", "CORES", "OWN", "WIN", "NW"))
    F_IN, H, D, OUT = cfg["F_IN"], cfg["H"], cfg["D"], cfg["OUT"]
    ROW0, ROW1, SPLIT = cfg["ROW0"], cfg["ROW1"], cfg["SPLIT"]
    NROWS = cfg["NROWS"]
    lo_slots, hi_slots, tpw = meta["lo_slots"], meta["hi_slots"], meta["tpw"]
    T_TILES, IDX_COLS = meta["t_tiles"], meta["idx_cols"]
    HD = H * D
    C0 = F_IN + 2 * H          # wcat0 cols (h | al_s | al_d)
    C1 = OUT + 2               # wcat1 cols (h1 | as1 | ad1), + OUT residual
    W1C = wcat1_cols = C1 + OUT
    KCH = (F_IN + 127) // 128  # contraction chunks (2)

    nc = bacc.Bacc("TRN2", target_bir_lowering=False, debug=False,
                   num_devices=CORES)

    # I/O
    xT = nc.dram_tensor("xT", [F_IN, OWN], F32, kind="ExternalInput")
    idx_in = nc.dram_tensor("idx", [128, IDX_COLS], dt.int16, kind="ExternalInput")
    dstc_in = nc.dram_tensor("dstc", [128, T_TILES], F32, kind="ExternalInput")
    wcat0_in = nc.dram_tensor("wcat0", [F_IN, F_IN + 2 * H + F_IN], F32, kind="ExternalInput")
    bias0_in = nc.dram_tensor("bias0", [128, KCH], F32, kind="ExternalInput")
    wcat1_in = nc.dram_tensor("wcat1", [HD, W1C], F32, kind="ExternalInput")
    biasr1_in = nc.dram_tensor("bias_row1", [1, W1C], F32, kind="ExternalInput")
    ident_in = nc.dram_tensor("ident", [128, 128], F32, kind="ExternalInput")
    iota_in = nc.dram_tensor("iota", [128, 128], F32, kind="ExternalInput")
    ones_in = nc.dram_tensor("ones_row", [1, 128], F32, kind="ExternalInput")
    out_own = nc.dram_tensor("out_own", [OWN, OUT], F32, kind="ExternalOutput")

    # internal DRAM
    tbl0_shard = nc.dram_tensor("tbl0_shard", [OWN, ROW0], F32)
    tbl1_shard = nc.dram_tensor("tbl1_shard", [OWN, ROW1], F32)
    aspace = "Shared" if CORES > 4 else "Local"
    reg0 = nc.dram_tensor("reg0", [NROWS, ROW0], F32, addr_space=aspace)
    reg1 = nc.dram_tensor("reg1", [NROWS, ROW1], F32, addr_space=aspace)

    rg = [list(range(CORES))]

    def win_size(w):
        return min(WIN, OWN - w * WIN)

    with tile.TileContext(nc) as tc, ExitStack() as ctx:
        const = ctx.enter_context(tc.tile_pool(name="const", bufs=1))
        persist = ctx.enter_context(tc.tile_pool(name="persist", bufs=1))

        WC0 = F_IN + 2 * H + F_IN
        wcat0_t = const.tile([128, KCH, WC0], F32)
        for k in range(KCH):
            nc.sync.dma_start(wcat0_t[:, k, :], wcat0_in[k * 128:(k + 1) * 128, :])
        bias0_t = const.tile([128, KCH], F32)
        nc.sync.dma_start(bias0_t[:], bias0_in[:])
        wcat1_t = const.tile([128, KCH, W1C], F32)
        for k in range(KCH):
            nc.sync.dma_start(wcat1_t[:, k, :], wcat1_in[k * 128:(k + 1) * 128, :])
        biasr1_t = const.tile([1, W1C], F32)
        nc.sync.dma_start(biasr1_t[:], biasr1_in[:])
        ident_t = const.tile([128, 128], F32)
        nc.sync.dma_start(ident_t[:], ident_in[:])
        if mmdt != F32:
            ident_r = const.tile([128, 128], mmdt)
            nc.vector.tensor_copy(ident_r[:], ident_t[:])
        else:
            ident_r = ident_t

        iota_t = const.tile([128, 128], F32)
        nc.sync.dma_start(iota_t[:], iota_in[:])
        ones_t = const.tile([1, 128], F32)
        nc.sync.dma_start(ones_t[:], ones_in[:])
        idx_t = persist.tile([128, IDX_COLS], dt.int16)
        nc.sync.dma_start(idx_t[:], idx_in[:])
        dstc_t = persist.tile([128, T_TILES], F32)
        nc.sync.dma_start(dstc_t[:], dstc_in[:])

        res0T_d = nc.dram_tensor("res0T_d", [KCH, 128, OWN], F32)
        for _rep in range(repeat):
            al_dst0 = persist.tile([128, NW, H], F32)
            al_dst1 = persist.tile([128, NW, 1], F32)
            res1 = persist.tile([128, NW, OUT], F32)


            # ---- setup: table0 shard rows + res0T -------------------------------
            with tc.tile_pool(name="setup_ps", bufs=2, space="PSUM") as sps, \
                 tc.tile_pool(name="setup_sb", bufs=3) as ssb, \
                 tc.tile_pool(name="xpool", bufs=1) as xp:
                xT_t = xp.tile([128, KCH, OWN], F32)
                for k in range(KCH):
                    nc.sync.dma_start(xT_t[:, k, :], xT[k * 128:(k + 1) * 128, :])
                for w in range(NW):
                    ws = win_size(w)
                    r0, r1 = w * WIN, w * WIN + ws
                    ps_a = sps.tile([128, C0], F32, tag="ps_a")
                    ps_b = sps.tile([128, F_IN], F32, tag="ps_b")
                    for k in range(KCH):
                        lhsT = xT_t[:, k, r0:r1]
                        nc.tensor.matmul(ps_a[:ws, :], lhsT, wcat0_t[:, k, :C0],
                                         start=(k == 0), stop=(k == KCH - 1))
                        nc.tensor.matmul(ps_b[:ws, :], lhsT, wcat0_t[:, k, C0:],
                                         start=(k == 0), stop=(k == KCH - 1))
                    stage = ssb.tile([128, ROW0], F32, tag="stage")
                    nc.vector.tensor_copy(stage[:ws, :F_IN + H], ps_a[:ws, :F_IN + H])
                    nc.vector.memset(stage[:ws, F_IN + H:], 0.0)
                    nc.vector.tensor_copy(al_dst0[:ws, w, :], ps_a[:ws, F_IN + H:F_IN + 2 * H])
                    nc.sync.dma_start(tbl0_shard[r0:r1, :], stage[:ws, :])
                # res0T: [F_IN, OWN] in KCH chunks of 128 partitions
                RT = 512
                for k in range(KCH):
                    for j0 in range(0, OWN, RT):
                        j1 = min(j0 + RT, OWN)
                        ps_r = sps.tile([128, RT], F32, tag="ps_r")
                        for kk in range(KCH):
                            nc.tensor.matmul(
                                ps_r[:, :j1 - j0],
                                wcat0_t[:, kk, C0 + k * 128:C0 + (k + 1) * 128],
                                xT_t[:, kk, j0:j1],
                                start=(kk == 0), stop=(kk == KCH - 1))
                        rstage = ssb.tile([128, RT], F32, tag="rstage")
                        nc.scalar.activation(rstage[:, :j1 - j0], ps_r[:, :j1 - j0],
                                             ACT.Identity, bias=bias0_t[:, k:k + 1])
                        nc.sync.dma_start(res0T_d[k, :, j0:j1], rstage[:, :j1 - j0])

            # sentinels for reg0 / reg1
            with tc.tile_pool(name="sent", bufs=1) as sp:
                s0 = sp.tile([1, ROW0], F32)
                nc.vector.memset(s0[:], 0.0)
                nc.vector.memset(s0[:, F_IN:F_IN + H], -1e30)
                nc.sync.dma_start(reg0[0:1, :], s0[:])
                nc.sync.dma_start(reg0[NROWS - 1:NROWS, :], s0[:])
                s1 = sp.tile([1, ROW1], F32)
                nc.vector.memset(s1[:], 0.0)
                nc.vector.memset(s1[:, OUT:OUT + 1], -1e30)
                nc.sync.dma_start(reg1[0:1, :], s1[:])
                nc.sync.dma_start(reg1[NROWS - 1:NROWS, :], s1[:])

            if CORES == 1:
                nc.sync.dma_start(reg0[1:N + 1, :], tbl0_shard[:])
            else:
                nc.gpsimd.collective_compute(
                    "AllGather", AL.bypass, replica_groups=rg,
                    ins=[tbl0_shard[:].opt()], outs=[reg0[1:N + 1, :].opt()])

            # ---- layer 0 windows ------------------------------------------------
            def layer(lidx, region, row_elems, feat, nh, al_dst_t, table_cfg):
                """Shared window loop for both layers.
                feat: gathered feature dim (F_IN or OUT); nh: heads (H or 1).
                table_cfg: None for layer1, else dict to emit table1+res1."""
                idx_off = 0
                tile_off = 0
                elem = row_elems
                with tc.tile_pool(name=f"G{lidx}", bufs=gbufs) as gp, \
                     tc.tile_pool(name=f"w{lidx}", bufs=wbufs) as wp, \
                     tc.tile_pool(name=f"v{lidx}", bufs=vbufs) as vp, \
                     tc.tile_pool(name=f"ps{lidx}", bufs=2, space="PSUM") as pp, \
                     tc.tile_pool(name=f"pw{lidx}", bufs=2, space="PSUM") as pw, \
                     tc.tile_pool(name=f"tail{lidx}", bufs=2) as tl:
                    for w in range(NW):
                        ws = win_size(w)
                        L, Hs = lo_slots[w], hi_slots[w]
                        nt = (L + Hs) // 128
                        G = gp.tile([128, nt, elem], F32, tag="G")
                        GCAP = 896  # SWDGE desc ring holds 1024 descs; stay under
                        for base, nsl, ap_in in ((0, L, region[:, :]),
                                                 (L, Hs, region[SPLIT + 1:, :])):
                            for c0 in range(0, nsl, GCAP):
                                csl = min(GCAP, nsl - c0)
                                s0 = base + c0
                                nc.gpsimd.dma_gather(
                                    out_ap=G[:, s0 // 128:(s0 + csl) // 128, :],
                                    in_ap=ap_in,
                                    idxs_ap=idx_t[:, idx_off + s0 // 16:
                                                  idx_off + (s0 + csl) // 16],
                                    num_idxs=csl, num_idxs_reg=csl, elem_size=elem)
                        idx_off += (L + Hs) // 16

                        ps_agg = pp.tile([128, feat + nh], F32, tag="agg")
                        SUB = sub
                        use_r = (mmdt != F32) and (feat + nh >= 256)
                        wdt = mmdt if use_r else F32
                        for b0 in range(0, nt, SUB):
                            nb = min(SUB, nt - b0)
                            # batched W2 indicators for this sub-batch
                            W2 = wp.tile([128, SUB, 128], wdt, tag="W2")
                            nc.vector.tensor_tensor(
                                W2[:, :nb, :],
                                iota_t[:].unsqueeze(1).broadcast_to((128, nb, 128)),
                                dstc_t[:, tile_off + b0:tile_off + b0 + nb]
                                    .unsqueeze(2).broadcast_to((128, nb, 128)),
                                AL.is_equal)
                            ps_ad = pw.tile([128, SUB, nh], F32, tag="ad")
                            for t in range(nb):
                                w1p = pw.tile([128, 128], wdt, tag="scr")
                                nc.tensor.transpose(w1p[:], W2[:, t, :],
                                                    ident_r[:] if use_r else ident_t[:])
                                W1 = vp.tile([128, 128], F32, tag="W1")
                                nc.scalar.copy(W1[:], w1p[:])
                                nc.tensor.matmul(ps_ad[:, t, :], W1[:ws, :],
                                                 al_dst_t[:ws, w, :],
                                                 start=True, stop=True)
                            Gb = G[:, b0:b0 + nb, :]
                            V = vp.tile([128, SUB, feat + nh],
                                        mmdt if use_r else F32, tag="V")
                            s_t = wp.tile([128, SUB, nh], F32, tag="s")
                            nc.vector.tensor_tensor(s_t[:, :nb, :],
                                                    Gb[:, :, feat:feat + nh],
                                                    ps_ad[:, :nb, :], AL.add)
                            nc.vector.scalar_tensor_tensor(
                                s_t[:, :nb, :], s_t[:, :nb, :], NEG_SLOPE,
                                s_t[:, :nb, :], AL.mult, AL.max)
                            nc.scalar.activation(V[:, :nb, feat:feat + nh],
                                                 s_t[:, :nb, :], ACT.Exp)
                            if nh == 1:
                                nc.vector.tensor_tensor(
                                    V[:, :nb, :feat], Gb[:, :, :feat],
                                    V[:, :nb, feat:feat + nh]
                                        .broadcast_to((128, nb, feat)),
                                    AL.mult)
                            else:
                                nc.vector.tensor_tensor(
                                    V[:, :nb, :feat].rearrange(
                                        "p t (k d) -> p t k d", k=nh),
                                    Gb[:, :, :feat].rearrange(
                                        "p t (k d) -> p t k d", k=nh),
                                    V[:, :nb, feat:feat + nh].unsqueeze(3)
                                        .broadcast_to((128, nb, nh, D)),
                                    AL.mult)
                            for t in range(nb):
                                gt = b0 + t
                                nc.tensor.matmul(ps_agg[:], W2[:, t, :],
                                                 V[:, t, :],
                                                 start=(gt == 0),
                                                 stop=(gt == nt - 1))
                        tile_off += nt

                        # window tail
                        den = tl.tile([128, nh], F32, tag="den")
                        nc.vector.tensor_scalar(den[:ws, :], ps_agg[:ws, feat:], 1e-16,
                                                None, AL.add)
                        rden = tl.tile([128, nh], F32, tag="rden")
                        nc.vector.reciprocal(rden[:ws, :], den[:ws, :])
                        o0 = tl.tile([128, feat], F32, tag="o0")
                        if nh == 1:
                            nc.vector.tensor_scalar(o0[:ws, :], ps_agg[:ws, :feat],
                                                    rden[:ws, :], None, AL.mult)
                        else:
                            nc.vector.tensor_tensor(
                                o0[:ws, :].rearrange("p (k d) -> p k d", k=nh),
                                ps_agg[:ws, :feat].rearrange("p (k d) -> p k d", k=nh),
                                rden[:ws, :].unsqueeze(2).broadcast_to((ws, nh, D)),
                                AL.mult)
                        yield w, ws, o0, tl, pw

            # layer 0 consumer: ELU(o0+res0T) -> h_outT; table1 rows
            gen = layer(0, reg0, ROW0, F_IN, H, al_dst0, None)
            for w, ws, o0, tl, pw in gen:
                r0 = w * WIN
                hT = tl.tile([128, KCH, 128], F32, tag="hT")
                res_w = tl.tile([128, KCH, 128], F32, tag="res_w")
                for k in range(KCH):
                    nc.sync.dma_start(res_w[:, k, :ws], res0T_d[k, :, r0:r0 + ws])
                tp = pw.tile([128, KCH, 128], F32, tag="scr")
                for k in range(KCH):
                    nc.tensor.transpose(tp[:, k, :ws], o0[:ws, k * 128:(k + 1) * 128],
                                        ident_t[:ws, :ws])
                # ELU(x + res): h = max(x,0) + exp(min(x,0)) - 1
                xr = tl.tile([128, KCH, 128], F32, tag="xr")
                nc.vector.tensor_tensor(xr[:, :, :ws], tp[:, :, :ws],
                                        res_w[:, :, :ws], AL.add)
                mn = tl.tile([128, KCH, 128], F32, tag="mn")
                nc.vector.tensor_scalar(mn[:, :, :ws], xr[:, :, :ws], 0.0, None, AL.min)
                nc.scalar.activation(mn[:, :, :ws], mn[:, :, :ws], ACT.Exp)
                nc.vector.tensor_scalar(xr[:, :, :ws], xr[:, :, :ws], 0.0, None, AL.max)
                nc.vector.tensor_tensor(xr[:, :, :ws], xr[:, :, :ws], mn[:, :, :ws], AL.add)
                nc.vector.tensor_scalar(hT[:, :, :ws], xr[:, :, :ws], 1.0, None,
                                        AL.subtract)
                # table1 rows: psum = hT.T @ wcat1 (+ ones*bias_row)
                ps_t1 = pw.tile([128, W1C], F32, tag="scr")
                for k in range(KCH):
                    nc.tensor.matmul(ps_t1[:ws, :], hT[:, k, :ws],
                                     wcat1_t[:, k, :],
                                     start=(k == 0), stop=False)
                nc.tensor.matmul(ps_t1[:ws, :], ones_t[:, :ws], biasr1_t[:],
                                 start=False, stop=True)
                st1 = tl.tile([128, ROW1], F32, tag="st1")
                nc.vector.tensor_copy(st1[:ws, :OUT + 1], ps_t1[:ws, :OUT + 1])
                nc.vector.memset(st1[:ws, OUT + 1:], 0.0)
                nc.vector.tensor_copy(al_dst1[:ws, w, :], ps_t1[:ws, OUT + 1:OUT + 2])
                nc.vector.tensor_copy(res1[:ws, w, :], ps_t1[:ws, OUT + 2:])
                nc.sync.dma_start(tbl1_shard[r0:r0 + ws, :], st1[:ws, :])

            if CORES == 1:
                nc.sync.dma_start(reg1[1:N + 1, :], tbl1_shard[:])
            else:
                nc.gpsimd.collective_compute(
                    "AllGather", AL.bypass, replica_groups=rg,
                    ins=[tbl1_shard[:].opt()], outs=[reg1[1:N + 1, :].opt()])

            # layer 1 consumer: +res1, log_softmax, out
            gen1 = layer(1, reg1, ROW1, OUT, 1, al_dst1, None)
            for w, ws, o0, tl, pw in gen1:
                r0 = w * WIN
                nc.vector.tensor_tensor(o0[:ws, :], o0[:ws, :], res1[:ws, w, :], AL.add)
                mxv = tl.tile([128, 1], F32, tag="mxv")
                nc.vector.reduce_max(mxv[:ws, :], o0[:ws, :], axis=mybir.AxisListType.X)
                nc.vector.tensor_scalar(o0[:ws, :], o0[:ws, :], mxv[:ws, :], None,
                                        AL.subtract)
                ev = tl.tile([128, OUT], F32, tag="ev")
                nc.scalar.activation(ev[:ws, :], o0[:ws, :], ACT.Exp)
                sv = tl.tile([128, 1], F32, tag="sv")
                nc.vector.reduce_sum(sv[:ws, :], ev[:ws, :], axis=mybir.AxisListType.X)
                lv = tl.tile([128, 1], F32, tag="lv")
                nc.scalar.activation(lv[:ws, :], sv[:ws, :], ACT.Ln)
                nc.vector.tensor_scalar(o0[:ws, :], o0[:ws, :], lv[:ws, :], None,
                                        AL.subtract)
                nc.sync.dma_start(out_own[r0:r0 + ws, :], o0[:ws, :])


    nc.compile()
    return nc




# ----------------------------------------------------------------- entrypoint

_CORES = 8

def kernel(x, edge_index, W0, a_src0, a_dst0, b0, Wr0, br0,
           W1, a_src1, a_dst1, b1, Wr1, br1):
    """Full-input GAT kernel: shards across 8 NeuronCores internally."""
    x = np.asarray(x)
    edge_index = np.asarray(edge_index)
    N, F_in = x.shape
    E = edge_index.shape[1]
    H, D = np.asarray(a_src0).shape
    OUT = np.asarray(a_src1).shape[1]
    cfg = make_cfg(N, E, F_in, H, D, OUT, _CORES)
    weights = dict(
        W0=np.asarray(W0, np.float32), a_src0=np.asarray(a_src0, np.float32),
        a_dst0=np.asarray(a_dst0, np.float32), b0=np.asarray(b0, np.float32),
        Wr0=np.asarray(Wr0, np.float32), br0=np.asarray(br0, np.float32),
        W1=np.asarray(W1, np.float32), a_src1=np.asarray(a_src1, np.float32),
        a_dst1=np.asarray(a_dst1, np.float32), b1=np.asarray(b1, np.float32),
        Wr1=np.asarray(Wr1, np.float32), br1=np.asarray(br1, np.float32))
    in_maps, meta = prepare(x.astype(np.float32), edge_index, weights, cfg)
    nc = build(cfg, meta, mmdt=dt.float32r)
    from concourse.bass_utils import run_bass_kernel_spmd
    res = run_bass_kernel_spmd(nc, in_maps, list(range(_CORES)))
    out = np.concatenate([res.results[c]["out_own"] for c in range(_CORES)],
                         axis=0).astype(np.float32)
    return out



# revision 9
# speedup vs baseline: 6.7473x; 6.7473x over previous
import sys
for _p in ("/opt/trn_rl_repo",):
    if _p not in sys.path:
        sys.path.insert(0, _p)
"""GAT 2-layer kernel for TRN2, 8-core dst-sharded — v2 "lane layout".

This environment has a large fixed per-instruction dispatch cost, so the
design minimizes instruction count:
  * nodes are relabeled (host) so each dst window's 128 nodes sit on 128
    SBUF partitions ("lanes"); a window's edges live at [lane, j] with
    j < K_w (degree-balanced relabeling keeps K_w ~ mean degree)
  * per-edge softmax weights and aggregation are pure DVE ops: al_dst add
    is a free-dim broadcast, the neighbor sum is one strided-AP reduce —
    no indicator matmuls, no per-tile transposes
  * int16 gather indices cannot span 50k table rows, so each edge block is
    fetched in an even/odd row-parity pass (the other parity reads a zero
    row) and the two passes are summed
Tables are bf16; weights/x are cast host-side.
"""

import numpy as np
import ml_dtypes
from contextlib import ExitStack

import concourse.bass as bass
import concourse.bacc as bacc
import concourse.mybir as mybir
import concourse.tile as tile

dt = mybir.dt
F32 = dt.float32
BF16 = dt.bfloat16
AL = mybir.AluOpType
ACT = mybir.ActivationFunctionType
BF = ml_dtypes.bfloat16

NEG_SLOPE = 0.2
_CORES = 8
WIN = 128
GCAP = 896
MAX_TILES = 60


def wrap_idx(iv):
    """[n*16] int -> [128, n] int16 gather layout (t -> (t%16, t//16))."""
    iv = np.asarray(iv, np.int64)
    assert len(iv) % 16 == 0
    cols = len(iv) // 16
    a = iv.reshape(cols, 16).T.astype(np.int16)
    return np.tile(a, (8, 1))


def make_cfg(N, E, F_in, H, D, OUT, cores):
    own = (N + cores - 1) // cores
    nw = (own + WIN - 1) // WIN
    ownp = nw * WIN
    cfg = dict(
        N=N, E=E, F_IN=F_in, H=H, D=D, OUT=OUT, CORES=cores, OWN=own,
        NW=nw, OWNP=ownp,
        ROW0=384, ROW1=256,            # bf16 elems per table row
        C0=F_in + 2 * H,               # wcat0 cols (h | al_s | al_d)
        C1=OUT + 2 + OUT,              # wcat1 cols (hW1 | als | ald | res)
        NROWS=4 + cores * ownp,        # [Ze, Zo, Pe, x] + nodes
    )
    assert cfg["NROWS"] % 2 == 0
    cfg["VROWS"] = cfg["NROWS"] // 2
    assert cfg["VROWS"] - 2 <= 32767
    return cfg


def prepare(x, edge_index, weights, cfg):
    N, E, C, OWN, OWNP, NW = (cfg[k] for k in
                              ("N", "E", "CORES", "OWN", "OWNP", "NW"))
    H, D, F_IN, OUT = cfg["H"], cfg["D"], cfg["F_IN"], cfg["OUT"]

    src = np.concatenate([np.asarray(edge_index[0]),
                          np.arange(N)]).astype(np.int64)
    dst = np.concatenate([np.asarray(edge_index[1]),
                          np.arange(N)]).astype(np.int64)

    # degree-balanced relabeling: node order[i] -> (core i%C, pos i//C)
    deg = np.bincount(dst, minlength=N)
    order = np.argsort(-deg, kind="stable")
    core_of = np.empty(N, np.int64)
    pos_of = np.empty(N, np.int64)
    core_of[order] = np.arange(N) % C
    pos_of[order] = np.arange(N) // C

    # per-window max lane count (shared across cores)
    cnt = np.zeros((C, OWNP), np.int64)
    np.add.at(cnt, (core_of[dst], pos_of[dst]), 1)
    Kw = cnt.reshape(C, NW, WIN).max(axis=(0, 2))

    # batches of windows with uniform K, capped at MAX_TILES j-tiles
    batches = []
    w = 0
    while w < NW:
        K = max(int(Kw[w]), 1)
        assert K <= MAX_TILES, f"window degree {K} exceeds MAX_TILES"
        nw_b = 1
        while w + nw_b < NW:
            K2 = max(K, int(Kw[w + nw_b]))
            if (nw_b + 1) * K2 > MAX_TILES:
                break
            K = K2
            nw_b += 1
        batches.append((w, nw_b, K))
        w += nw_b

    slot0_w = np.zeros(NW, np.int64)
    base = 0
    for (w0, nw_b, K) in batches:
        for wl in range(nw_b):
            slot0_w[w0 + wl] = base + wl * K * WIN
        base += nw_b * K * WIN
    totslot = base
    assert totslot % 16 == 0

    # per-edge slot assignment: j = rank within (core,pos) group
    ec, ep = core_of[dst], pos_of[dst]
    order_e = np.lexsort((src, ep, ec))
    src_s, ec_s, ep_s = src[order_e], ec[order_e], ep[order_e]
    grp = ec_s * OWNP + ep_s
    starts = np.searchsorted(grp, np.arange(C * OWNP))
    j_of = np.arange(len(grp)) - starts[grp]
    wd = ep_s // WIN
    lane = ep_s % WIN
    slot = slot0_w[wd] + j_of * WIN + lane
    srow = 4 + core_of[src_s] * OWNP + pos_of[src_s]   # table row of source
    even = (srow % 2 == 0)

    in_maps = []
    for c in range(C):
        m = ec_s == c
        rowE = np.full(totslot, 1, np.int64)   # default: PAD row (view idx 1)
        rowO = np.zeros(totslot, np.int64)     # default: zero row
        sl, sr, ev = slot[m], srow[m], even[m]
        rowE[sl[ev]] = sr[ev] // 2
        rowO[sl[ev]] = 0
        rowE[sl[~ev]] = 0
        rowO[sl[~ev]] = (sr[~ev] - 1) // 2
        nodes = np.full(OWNP, -1, np.int64)
        mine = core_of == c
        nodes[pos_of[mine]] = np.where(mine)[0]
        xT = np.zeros((F_IN, OWNP), BF)
        valid = nodes >= 0
        xT[:, valid] = np.asarray(x, np.float32)[nodes[valid]].T.astype(BF)
        in_maps.append(dict(
            xT=xT,
            idxE=wrap_idx(rowE), idxO=wrap_idx(rowO),
        ))

    # shared constants
    W0 = weights["W0"]; a_s0 = weights["a_src0"]; a_d0 = weights["a_dst0"]
    Wr0 = weights["Wr0"]; W1 = weights["W1"]; a_s1 = weights["a_src1"]
    a_d1 = weights["a_dst1"]; Wr1 = weights["Wr1"]
    blk0s = np.zeros((H * D, H), np.float32)
    blk0d = np.zeros((H * D, H), np.float32)
    for k in range(H):
        blk0s[k * D:(k + 1) * D, k] = a_s0[k]
        blk0d[k * D:(k + 1) * D, k] = a_d0[k]
    wcat0 = np.concatenate([W0, W0 @ blk0s, W0 @ blk0d], axis=1).astype(BF)
    wcat1 = np.concatenate([W1, W1 @ a_s1.reshape(-1, 1),
                            W1 @ a_d1.reshape(-1, 1), Wr1], axis=1).astype(BF)
    bias0 = (weights["b0"] + weights["br0"]).astype(np.float32)   # [256]
    bias_row1 = np.zeros((1, cfg["C1"]), BF)
    bias_row1[0, OUT + 2:] = (weights["b1"] + weights["br1"]).astype(BF)
    consts = dict(
        wcat0=wcat0, wr0=np.asarray(Wr0, np.float32).astype(BF),
        wcat1=wcat1, bias_row1=bias_row1,
        bias0=np.ascontiguousarray(bias0.reshape(2, 128).T),  # [128, 2]
    )
    for m in in_maps:
        m.update(consts)
    meta = dict(batches=batches, totslot=totslot,
                core_of=core_of, pos_of=pos_of)
    return in_maps, meta


def build(cfg, meta, repeat=1, abl=()):
    NOCOLL = "nocoll" in abl
    NOGATHER = "nogather" in abl
    N, C, OWN, OWNP, NW = (cfg[k] for k in
                           ("N", "CORES", "OWN", "OWNP", "NW"))
    F_IN, H, D, OUT = cfg["F_IN"], cfg["H"], cfg["D"], cfg["OUT"]
    ROW0, ROW1, C0, C1 = cfg["ROW0"], cfg["ROW1"], cfg["C0"], cfg["C1"]
    VROWS = cfg["VROWS"]
    batches = meta["batches"]
    TOTSLOT = meta["totslot"]
    KCH = (F_IN + 127) // 128   # 2

    nc = bacc.Bacc("TRN2", target_bir_lowering=False, debug=False,
                   num_devices=C)

    xT_in = nc.dram_tensor("xT", [F_IN, OWNP], BF16, kind="ExternalInput")
    idxE_in = nc.dram_tensor("idxE", [128, TOTSLOT // 16], dt.int16,
                             kind="ExternalInput")
    idxO_in = nc.dram_tensor("idxO", [128, TOTSLOT // 16], dt.int16,
                             kind="ExternalInput")
    wcat0_in = nc.dram_tensor("wcat0", [F_IN, C0], BF16, kind="ExternalInput")
    wr0_in = nc.dram_tensor("wr0", [F_IN, F_IN], BF16, kind="ExternalInput")
    wcat1_in = nc.dram_tensor("wcat1", [F_IN, C1], BF16, kind="ExternalInput")
    biasr1_in = nc.dram_tensor("bias_row1", [1, C1], BF16, kind="ExternalInput")
    bias0_in = nc.dram_tensor("bias0", [128, KCH], F32, kind="ExternalInput")
    out_own = nc.dram_tensor("out_own", [OWNP, OUT], F32, kind="ExternalOutput")

    tbl0_shard = nc.dram_tensor("tbl0_shard", [OWNP, ROW0], BF16)
    tbl1_shard = nc.dram_tensor("tbl1_shard", [OWNP, ROW1], BF16)
    reg0 = nc.dram_tensor("reg0", [VROWS, 2 * ROW0], BF16, addr_space="Shared")
    reg1 = nc.dram_tensor("reg1", [VROWS, 2 * ROW1], BF16, addr_space="Shared")
    rg = [list(range(C))]

    with tile.TileContext(nc) as tc, ExitStack() as ctx:
        const = ctx.enter_context(tc.tile_pool(name="const", bufs=1))
        wcat0_t = const.tile([128, KCH, C0], BF16)
        nc.sync.dma_start(wcat0_t[:], wcat0_in[:, :].rearrange(
            "(k p) c -> p k c", p=128))
        wr0_t = const.tile([128, KCH, F_IN], BF16)
        nc.sync.dma_start(wr0_t[:], wr0_in[:, :].rearrange(
            "(k p) c -> p k c", p=128))
        wcat1_t = const.tile([128, KCH, C1], BF16)
        nc.sync.dma_start(wcat1_t[:], wcat1_in[:, :].rearrange(
            "(k p) c -> p k c", p=128))
        biasr1_t = const.tile([1, C1], BF16)
        nc.sync.dma_start(biasr1_t[:], biasr1_in[:])
        bias0_t = const.tile([128, KCH], F32)
        nc.sync.dma_start(bias0_t[:], bias0_in[:])
        ones_t = const.tile([1, 128], BF16)
        nc.vector.memset(ones_t[:], 1.0)
        idxE_t = const.tile([128, TOTSLOT // 16], dt.int16)
        nc.sync.dma_start(idxE_t[:], idxE_in[:])
        idxO_t = const.tile([128, TOTSLOT // 16], dt.int16)
        nc.sync.dma_start(idxO_t[:], idxO_in[:])

        persist = ctx.enter_context(tc.tile_pool(name="persist", bufs=1))
        res0T = persist.tile([128, KCH, OWNP], BF16)
        h1T = persist.tile([128, KCH, OWNP], BF16)
        ad0 = persist.tile([128, NW, H], BF16)
        ad1 = persist.tile([128, NW, 1 + OUT], BF16)

        for _rep in range(repeat):
            # ---------------- setup: table0 rows, res0T, sentinels ----------
            with tc.tile_pool(name="xp", bufs=1) as xp, \
                 tc.tile_pool(name="sps", bufs=2, space="PSUM") as sps, \
                 tc.tile_pool(name="ssb", bufs=2) as ssb:
                xT_t = xp.tile([128, KCH, OWNP], BF16)
                nc.sync.dma_start(xT_t[:], xT_in[:, :].rearrange(
                    "(k p) n -> p k n", p=128))
                for w in range(NW):
                    ps = sps.tile([128, C0], F32, tag="t0")
                    for k in range(KCH):
                        nc.tensor.matmul(ps[:], xT_t[:, k, w * 128:(w + 1) * 128],
                                         wcat0_t[:, k, :],
                                         start=(k == 0), stop=(k == KCH - 1))
                    st = ssb.tile([128, C0], BF16, tag="st0")
                    nc.scalar.copy(st[:], ps[:])
                    nc.sync.dma_start(tbl0_shard[w * 128:(w + 1) * 128, :C0],
                                      st[:])
                RT = 512
                for fc in range(KCH):
                    for n0 in range(0, OWNP, RT):
                        n1 = min(n0 + RT, OWNP)
                        ps = sps.tile([128, RT], F32, tag="r0")
                        for k in range(KCH):
                            nc.tensor.matmul(
                                ps[:, :n1 - n0],
                                wr0_t[:, k, fc * 128:(fc + 1) * 128],
                                xT_t[:, k, n0:n1],
                                start=(k == 0), stop=(k == KCH - 1))
                        nc.scalar.activation(res0T[:, fc, n0:n1],
                                             ps[:, :n1 - n0],
                                             ACT.Identity,
                                             bias=bias0_t[:, fc:fc + 1])
                # sentinels: view rows 0..1  (= table rows 0..3).
                # row 0 = zeros; row 1 starts with the PAD row (al = -1e30).
                sz = ssb.tile([1, 2 * ROW0], BF16, tag="sz")
                nc.vector.memset(sz[:], 0.0)
                sp = ssb.tile([1, 2 * ROW0], BF16, tag="sp")
                nc.vector.memset(sp[:], 0.0)
                nc.vector.memset(sp[:, F_IN:F_IN + H], -1e30)
                nc.sync.dma_start(reg0[0:1, :], sz[:])
                nc.sync.dma_start(reg0[1:2, :], sp[:])
                s1z = ssb.tile([1, 2 * ROW1], BF16, tag="s1z")
                nc.vector.memset(s1z[:], 0.0)
                s1p = ssb.tile([1, 2 * ROW1], BF16, tag="s1p")
                nc.vector.memset(s1p[:], 0.0)
                nc.vector.memset(s1p[:, OUT:OUT + 1], -1e30)
                nc.sync.dma_start(reg1[0:1, :], s1z[:])
                nc.sync.dma_start(reg1[1:2, :], s1p[:])

            nc.sync.dma_start(
                ad0[:], tbl0_shard[:, F_IN + H:F_IN + 2 * H].rearrange(
                    "(w p) e -> p w e", p=128))

            if NOCOLL:
                nc.sync.dma_start(
                    reg0[2:2 + OWNP // 2, :],
                    tbl0_shard[:, :].rearrange("(v t) e -> v (t e)", t=2))
            else:
                nc.gpsimd.collective_compute(
                    "AllGather", AL.bypass, replica_groups=rg,
                    ins=[tbl0_shard[:].opt()], outs=[reg0[2:, :].opt()])

            # ---------------- layer 0 ----------------
            def gather_batch(gp, go, idx_off, slots, elem, regv_e, regv_o,
                             step):
                G = gp.tile([128, slots // 128, elem], BF16, tag="G")
                if NOGATHER:
                    nc.vector.memset(G[:], 0.0)
                    return G
                for c0 in range(0, slots, GCAP):
                    csl = min(GCAP, slots - c0)
                    Go = go.tile([128, GCAP // 128, elem], BF16, tag="Go")
                    nc.gpsimd.dma_gather(
                        out_ap=G[:, c0 // 128:(c0 + csl) // 128, :],
                        in_ap=regv_e,
                        idxs_ap=idxE_t[:, (idx_off + c0) // 16:
                                       (idx_off + c0 + csl) // 16],
                        num_idxs=csl, num_idxs_reg=csl, elem_size=elem,
                        elem_step=step)
                    nc.gpsimd.dma_gather(
                        out_ap=Go[:, :csl // 128, :],
                        in_ap=regv_o,
                        idxs_ap=idxO_t[:, (idx_off + c0) // 16:
                                       (idx_off + c0 + csl) // 16],
                        num_idxs=csl, num_idxs_reg=csl, elem_size=elem,
                        elem_step=step)
                    nc.vector.tensor_tensor(
                        G[:, c0 // 128:(c0 + csl) // 128, :],
                        G[:, c0 // 128:(c0 + csl) // 128, :],
                        Go[:, :csl // 128, :], AL.add)
                return G

            idx_off = 0
            with tc.tile_pool(name="g0", bufs=1) as gp, \
                 tc.tile_pool(name="go0", bufs=2) as go, \
                 tc.tile_pool(name="wk0", bufs=2) as wk:
                for (w0, nw_b, K) in batches:
                    slots = nw_b * K * 128
                    G = gather_batch(gp, go, idx_off, slots, ROW0,
                                     reg0[:, :ROW0], reg0[:, ROW0:],
                                     2 * ROW0)
                    idx_off += slots
                    Gv = G[:].rearrange("p (w k) e -> p w k e", w=nw_b)
                    s = wk.tile([128, nw_b, K, H], F32, tag="s")
                    nc.vector.tensor_tensor(
                        s[:], Gv[:, :, :, F_IN:F_IN + H],
                        ad0[:, w0:w0 + nw_b, :].unsqueeze(2)
                            .broadcast_to((128, nw_b, K, H)), AL.add)
                    nc.vector.scalar_tensor_tensor(s[:], s[:], NEG_SLOPE,
                                                   s[:], AL.mult, AL.max)
                    # per-lane max-shift: keeps exp args <= 0 (softmax
                    # shift-invariant; ACT exp is most accurate there)
                    mx = wk.tile([128, nw_b, H], F32, tag="mx")
                    nc.vector.reduce_max(
                        mx[:].unsqueeze(3),
                        s[:].rearrange("p w k h -> p w h k"),
                        axis=mybir.AxisListType.X)
                    nc.vector.tensor_tensor(
                        s[:], s[:],
                        mx[:].unsqueeze(2).broadcast_to((128, nw_b, K, H)),
                        AL.subtract)
                    Ex = wk.tile([128, nw_b, K, H], BF16, tag="E")
                    nc.scalar.activation(Ex[:], s[:], ACT.Exp)
                    nc.vector.tensor_tensor(
                        G[:, :, :F_IN].rearrange("p m (h d) -> p m h d", h=H),
                        G[:, :, :F_IN].rearrange("p m (h d) -> p m h d", h=H),
                        Ex[:].rearrange("p w k h -> p (w k) h").unsqueeze(3)
                            .broadcast_to((128, nw_b * K, H, D)), AL.mult)
                    nc.vector.tensor_copy(Gv[:, :, :, F_IN:F_IN + H], Ex[:])
                    U = wk.tile([128, nw_b, F_IN + H], F32, tag="U")
                    nc.vector.reduce_sum(
                        U[:].unsqueeze(3),
                        Gv[:, :, :, :F_IN + H].rearrange("p w k e -> p w e k"),
                        axis=mybir.AxisListType.X)
                    nc.vector.tensor_scalar(U[:, :, F_IN:], U[:, :, F_IN:],
                                            1e-16, None, AL.add)
                    rcp = wk.tile([128, nw_b, H], F32, tag="rcp")
                    nc.vector.reciprocal(rcp[:], U[:, :, F_IN:])
                    o0 = wk.tile([128, nw_b, F_IN], F32, tag="o0")
                    nc.vector.tensor_tensor(
                        o0[:].rearrange("p w (h d) -> p w h d", h=H),
                        U[:, :, :F_IN].rearrange("p w (h d) -> p w h d", h=H),
                        rcp[:].unsqueeze(3).broadcast_to((128, nw_b, H, D)),
                        AL.mult)
                    rT = wk.tile([128, nw_b, KCH, 128], BF16, tag="rT")
                    for wl in range(nw_b):
                        for k in range(KCH):
                            nc.sync.dma_start(
                                rT[:, wl, k, :],
                                res0T[:, k, (w0 + wl) * 128:(w0 + wl + 1) * 128],
                                transpose=True)
                    nc.vector.tensor_tensor(
                        o0[:], o0[:],
                        rT[:].rearrange("p w k e -> p w (k e)"), AL.add)
                    # ELU
                    mn = wk.tile([128, nw_b, F_IN], F32, tag="mn")
                    nc.vector.tensor_scalar(mn[:], o0[:], 0.0, None, AL.min)
                    nc.scalar.activation(mn[:], mn[:], ACT.Exp)
                    nc.vector.tensor_scalar(o0[:], o0[:], 0.0, None, AL.max)
                    nc.vector.tensor_tensor(o0[:], o0[:], mn[:], AL.add)
                    h1 = wk.tile([128, nw_b, F_IN], BF16, tag="h1")
                    nc.vector.tensor_scalar(h1[:], o0[:], 1.0, None,
                                            AL.subtract)
                    for wl in range(nw_b):
                        for k in range(KCH):
                            nc.sync.dma_start(
                                h1T[:, k, (w0 + wl) * 128:(w0 + wl + 1) * 128],
                                h1[:, wl, k * 128:(k + 1) * 128],
                                transpose=True)

            # ---------------- table1 rows ----------------
            with tc.tile_pool(name="t1ps", bufs=2, space="PSUM") as tps, \
                 tc.tile_pool(name="t1sb", bufs=2) as tsb:
                for w in range(NW):
                    ps = tps.tile([128, C1], F32, tag="t1")
                    for k in range(KCH):
                        nc.tensor.matmul(ps[:], h1T[:, k, w * 128:(w + 1) * 128],
                                         wcat1_t[:, k, :],
                                         start=(k == 0), stop=False)
                    nc.tensor.matmul(ps[:], ones_t[:], biasr1_t[:],
                                     start=False, stop=True)
                    st = tsb.tile([128, C1], BF16, tag="st1")
                    nc.scalar.copy(st[:], ps[:])
                    nc.sync.dma_start(tbl1_shard[w * 128:(w + 1) * 128, :C1],
                                      st[:])
            nc.sync.dma_start(
                ad1[:], tbl1_shard[:, OUT + 1:2 * OUT + 2].rearrange(
                    "(w p) e -> p w e", p=128))

            if NOCOLL:
                nc.sync.dma_start(
                    reg1[2:2 + OWNP // 2, :],
                    tbl1_shard[:, :].rearrange("(v t) e -> v (t e)", t=2))
            else:
                nc.gpsimd.collective_compute(
                    "AllGather", AL.bypass, replica_groups=rg,
                    ins=[tbl1_shard[:].opt()], outs=[reg1[2:, :].opt()])

            # ---------------- layer 1 ----------------
            GE1 = 128   # gather elems (covers hW1|als)
            idx_off = 0
            with tc.tile_pool(name="g1", bufs=1) as gp, \
                 tc.tile_pool(name="go1", bufs=2) as go, \
                 tc.tile_pool(name="wk1", bufs=2) as wk:
                for (w0, nw_b, K) in batches:
                    slots = nw_b * K * 128
                    G = gather_batch(gp, go, idx_off, slots, GE1,
                                     reg1[:, :GE1], reg1[:, ROW1:ROW1 + GE1],
                                     2 * ROW1)
                    idx_off += slots
                    Gv = G[:].rearrange("p (w k) e -> p w k e", w=nw_b)
                    s = wk.tile([128, nw_b, K, 1], F32, tag="s")
                    nc.vector.tensor_tensor(
                        s[:], Gv[:, :, :, OUT:OUT + 1],
                        ad1[:, w0:w0 + nw_b, 0:1].unsqueeze(2)
                            .broadcast_to((128, nw_b, K, 1)), AL.add)
                    nc.vector.scalar_tensor_tensor(s[:], s[:], NEG_SLOPE,
                                                   s[:], AL.mult, AL.max)
                    mx = wk.tile([128, nw_b, 1], F32, tag="mx")
                    nc.vector.reduce_max(
                        mx[:].unsqueeze(3),
                        s[:].rearrange("p w k h -> p w h k"),
                        axis=mybir.AxisListType.X)
                    nc.vector.tensor_tensor(
                        s[:], s[:],
                        mx[:].unsqueeze(2).broadcast_to((128, nw_b, K, 1)),
                        AL.subtract)
                    Ex = wk.tile([128, nw_b, K, 1], BF16, tag="E")
                    nc.scalar.activation(Ex[:], s[:], ACT.Exp)
                    nc.vector.tensor_tensor(
                        G[:, :, :OUT], G[:, :, :OUT],
                        Ex[:].rearrange("p w k h -> p (w k) h")
                            .broadcast_to((128, nw_b * K, OUT)), AL.mult)
                    nc.vector.tensor_copy(Gv[:, :, :, OUT:OUT + 1], Ex[:])
                    U = wk.tile([128, nw_b, OUT + 1], F32, tag="U")
                    nc.vector.reduce_sum(
                        U[:].unsqueeze(3),
                        Gv[:, :, :, :OUT + 1].rearrange("p w k e -> p w e k"),
                        axis=mybir.AxisListType.X)
                    nc.vector.tensor_scalar(U[:, :, OUT:], U[:, :, OUT:],
                                            1e-16, None, AL.add)
                    rcp = wk.tile([128, nw_b, 1], F32, tag="rcp")
                    nc.vector.reciprocal(rcp[:], U[:, :, OUT:])
                    o = wk.tile([128, nw_b, OUT], F32, tag="o")
                    nc.vector.tensor_tensor(
                        o[:], U[:, :, :OUT],
                        rcp[:].broadcast_to((128, nw_b, OUT)), AL.mult)
                    nc.vector.tensor_tensor(o[:], o[:],
                                            ad1[:, w0:w0 + nw_b, 1:],
                                            AL.add)
                    # log_softmax with max-shift
                    mxo = wk.tile([128, nw_b, 1], F32, tag="mxo")
                    nc.vector.reduce_max(mxo[:], o[:], axis=mybir.AxisListType.X)
                    nc.vector.tensor_tensor(
                        o[:], o[:], mxo[:].broadcast_to((128, nw_b, OUT)),
                        AL.subtract)
                    ev = wk.tile([128, nw_b, OUT], F32, tag="ev")
                    nc.scalar.activation(ev[:], o[:], ACT.Exp)
                    sv = wk.tile([128, nw_b, 1], F32, tag="sv")
                    nc.vector.reduce_sum(sv[:], ev[:], axis=mybir.AxisListType.X)
                    nc.scalar.activation(sv[:], sv[:], ACT.Ln)
                    nc.vector.tensor_tensor(
                        o[:], o[:], sv[:].broadcast_to((128, nw_b, OUT)),
                        AL.subtract)
                    nc.sync.dma_start(
                        out_own[w0 * 128:(w0 + nw_b) * 128, :].rearrange(
                            "(w p) e -> p w e", p=128), o[:])

    nc.compile()
    return nc


# ----------------------------------------------------------------- entrypoint

def kernel(x, edge_index, W0, a_src0, a_dst0, b0, Wr0, br0,
           W1, a_src1, a_dst1, b1, Wr1, br1):
    """Full-input GAT kernel: shards across 8 NeuronCores internally."""
    x = np.asarray(x)
    edge_index = np.asarray(edge_index)
    N, F_in = x.shape
    E = edge_index.shape[1]
    H, D = np.asarray(a_src0).shape
    OUT = np.asarray(a_src1).shape[1]
    cfg = make_cfg(N, E, F_in, H, D, OUT, _CORES)
    weights = dict(
        W0=np.asarray(W0, np.float32), a_src0=np.asarray(a_src0, np.float32),
        a_dst0=np.asarray(a_dst0, np.float32), b0=np.asarray(b0, np.float32),
        Wr0=np.asarray(Wr0, np.float32), br0=np.asarray(br0, np.float32),
        W1=np.asarray(W1, np.float32), a_src1=np.asarray(a_src1, np.float32),
        a_dst1=np.asarray(a_dst1, np.float32), b1=np.asarray(b1, np.float32),
        Wr1=np.asarray(Wr1, np.float32), br1=np.asarray(br1, np.float32))
    in_maps, meta = prepare(x.astype(np.float32), edge_index, weights, cfg)
    nc = build(cfg, meta)
    from concourse.bass_utils import run_bass_kernel_spmd
    res = run_bass_kernel_spmd(nc, in_maps, list(range(_CORES)))
    core_of, pos_of = meta["core_of"], meta["pos_of"]
    per_core = [np.asarray(res.results[c]["out_own"], np.float32)
                for c in range(_CORES)]
    stacked = np.stack(per_core)                       # [C, OWNP, OUT]
    out = stacked[core_of, pos_of]                     # [N, OUT]
    return out


# revision 11
# speedup vs baseline: 7.2029x; 1.0675x over previous
import sys
for _p in ("/opt/trn_rl_repo",):
    if _p not in sys.path:
        sys.path.insert(0, _p)
"""GAT 2-layer kernel for TRN2, 8-core dst-sharded — v2 "lane layout".

This environment has a large fixed per-instruction dispatch cost, so the
design minimizes instruction count:
  * nodes are relabeled (host) so each dst window's 128 nodes sit on 128
    SBUF partitions ("lanes"); a window's edges live at [lane, j] with
    j < K_w (degree-balanced relabeling keeps K_w ~ mean degree)
  * per-edge softmax weights and aggregation are pure DVE ops: al_dst add
    is a free-dim broadcast, the neighbor sum is one strided-AP reduce —
    no indicator matmuls, no per-tile transposes
  * int16 gather indices cannot span 50k table rows, so each edge block is
    fetched in an even/odd row-parity pass (the other parity reads a zero
    row) and the two passes are summed
Tables are bf16; weights/x are cast host-side.
"""

import numpy as np
import ml_dtypes
from contextlib import ExitStack

import concourse.bass as bass
import concourse.bacc as bacc
import concourse.mybir as mybir
import concourse.tile as tile

dt = mybir.dt
F32 = dt.float32
BF16 = dt.bfloat16
AL = mybir.AluOpType
ACT = mybir.ActivationFunctionType
BF = ml_dtypes.bfloat16

NEG_SLOPE = 0.2
_CORES = 8
WIN = 128
GCAP = 896
MAX_TILES = 60


def wrap_idx(iv):
    """[n*16] int -> [128, n] int16 gather layout (t -> (t%16, t//16))."""
    iv = np.asarray(iv, np.int64)
    assert len(iv) % 16 == 0
    cols = len(iv) // 16
    a = iv.reshape(cols, 16).T.astype(np.int16)
    return np.tile(a, (8, 1))


def make_cfg(N, E, F_in, H, D, OUT, cores):
    own = (N + cores - 1) // cores
    nw = (own + WIN - 1) // WIN
    ownp = nw * WIN
    cfg = dict(
        N=N, E=E, F_IN=F_in, H=H, D=D, OUT=OUT, CORES=cores, OWN=own,
        NW=nw, OWNP=ownp,
        ROW0=384, ROW1=256,            # bf16 elems per table row
        C0=F_in + 2 * H,               # wcat0 cols (h | al_s | al_d)
        C1=OUT + 2 + OUT,              # wcat1 cols (hW1 | als | ald | res)
        NROWS=4 + cores * ownp,        # [Ze, Zo, Pe, x] + nodes
    )
    assert cfg["NROWS"] % 2 == 0
    cfg["VROWS"] = cfg["NROWS"] // 2
    assert cfg["VROWS"] - 2 <= 32767
    return cfg


def prepare(x, edge_index, weights, cfg):
    N, E, C, OWN, OWNP, NW = (cfg[k] for k in
                              ("N", "E", "CORES", "OWN", "OWNP", "NW"))
    H, D, F_IN, OUT = cfg["H"], cfg["D"], cfg["F_IN"], cfg["OUT"]

    src = np.concatenate([np.asarray(edge_index[0]),
                          np.arange(N)]).astype(np.int64)
    dst = np.concatenate([np.asarray(edge_index[1]),
                          np.arange(N)]).astype(np.int64)

    # degree-balanced relabeling: node order[i] -> (core i%C, pos i//C)
    deg = np.bincount(dst, minlength=N)
    order = np.argsort(-deg, kind="stable")
    core_of = np.empty(N, np.int64)
    pos_of = np.empty(N, np.int64)
    core_of[order] = np.arange(N) % C
    pos_of[order] = np.arange(N) // C

    # per-window max lane count (shared across cores)
    cnt = np.zeros((C, OWNP), np.int64)
    np.add.at(cnt, (core_of[dst], pos_of[dst]), 1)
    Kw = cnt.reshape(C, NW, WIN).max(axis=(0, 2))

    # batches of windows with uniform K, capped at MAX_TILES j-tiles
    batches = []
    w = 0
    while w < NW:
        K = max(int(Kw[w]), 1)
        assert K <= MAX_TILES, f"window degree {K} exceeds MAX_TILES"
        nw_b = 1
        while w + nw_b < NW:
            K2 = max(K, int(Kw[w + nw_b]))
            if (nw_b + 1) * K2 > MAX_TILES:
                break
            K = K2
            nw_b += 1
        batches.append((w, nw_b, K))
        w += nw_b

    slot0_w = np.zeros(NW, np.int64)
    base = 0
    for (w0, nw_b, K) in batches:
        for wl in range(nw_b):
            slot0_w[w0 + wl] = base + wl * K * WIN
        base += nw_b * K * WIN
    totslot = base
    assert totslot % 16 == 0

    # per-edge slot assignment: j = rank within (core,pos) group
    ec, ep = core_of[dst], pos_of[dst]
    order_e = np.lexsort((src, ep, ec))
    src_s, ec_s, ep_s = src[order_e], ec[order_e], ep[order_e]
    grp = ec_s * OWNP + ep_s
    starts = np.searchsorted(grp, np.arange(C * OWNP))
    j_of = np.arange(len(grp)) - starts[grp]
    wd = ep_s // WIN
    lane = ep_s % WIN
    slot = slot0_w[wd] + j_of * WIN + lane
    srow = 4 + core_of[src_s] * OWNP + pos_of[src_s]   # table row of source
    even = (srow % 2 == 0)

    in_maps = []
    for c in range(C):
        m = ec_s == c
        rowE = np.full(totslot, 1, np.int64)   # default: PAD row (view idx 1)
        rowO = np.zeros(totslot, np.int64)     # default: zero row
        sl, sr, ev = slot[m], srow[m], even[m]
        rowE[sl[ev]] = sr[ev] // 2
        rowO[sl[ev]] = 0
        rowE[sl[~ev]] = 0
        rowO[sl[~ev]] = (sr[~ev] - 1) // 2
        nodes = np.full(OWNP, -1, np.int64)
        mine = core_of == c
        nodes[pos_of[mine]] = np.where(mine)[0]
        xT = np.zeros((F_IN, OWNP), BF)
        valid = nodes >= 0
        xT[:, valid] = np.asarray(x, np.float32)[nodes[valid]].T.astype(BF)
        in_maps.append(dict(
            xT=xT,
            idxE=wrap_idx(rowE), idxO=wrap_idx(rowO),
        ))

    # shared constants
    W0 = weights["W0"]; a_s0 = weights["a_src0"]; a_d0 = weights["a_dst0"]
    Wr0 = weights["Wr0"]; W1 = weights["W1"]; a_s1 = weights["a_src1"]
    a_d1 = weights["a_dst1"]; Wr1 = weights["Wr1"]
    blk0s = np.zeros((H * D, H), np.float32)
    blk0d = np.zeros((H * D, H), np.float32)
    for k in range(H):
        blk0s[k * D:(k + 1) * D, k] = a_s0[k]
        blk0d[k * D:(k + 1) * D, k] = a_d0[k]
    wcat0 = np.concatenate([W0, W0 @ blk0s, W0 @ blk0d], axis=1).astype(BF)
    wcat1 = np.concatenate([W1, W1 @ a_s1.reshape(-1, 1),
                            W1 @ a_d1.reshape(-1, 1), Wr1], axis=1).astype(BF)
    bias0 = (weights["b0"] + weights["br0"]).astype(np.float32)   # [256]
    bias_row1 = np.zeros((1, cfg["C1"]), BF)
    bias_row1[0, OUT + 2:] = (weights["b1"] + weights["br1"]).astype(BF)
    consts = dict(
        wcat0=wcat0, wr0=np.asarray(Wr0, np.float32).astype(BF),
        wcat1=wcat1, bias_row1=bias_row1,
        bias0=np.ascontiguousarray(bias0.reshape(2, 128).T),  # [128, 2]
    )
    for m in in_maps:
        m.update(consts)
    meta = dict(batches=batches, totslot=totslot,
                core_of=core_of, pos_of=pos_of)
    return in_maps, meta


def build(cfg, meta, repeat=1, abl=()):
    NOCOLL = "nocoll" in abl
    NOGATHER = "nogather" in abl
    N, C, OWN, OWNP, NW = (cfg[k] for k in
                           ("N", "CORES", "OWN", "OWNP", "NW"))
    F_IN, H, D, OUT = cfg["F_IN"], cfg["H"], cfg["D"], cfg["OUT"]
    ROW0, ROW1, C0, C1 = cfg["ROW0"], cfg["ROW1"], cfg["C0"], cfg["C1"]
    VROWS = cfg["VROWS"]
    batches = meta["batches"]
    TOTSLOT = meta["totslot"]
    KCH = (F_IN + 127) // 128   # 2

    nc = bacc.Bacc("TRN2", target_bir_lowering=False, debug=False,
                   num_devices=C)

    xT_in = nc.dram_tensor("xT", [F_IN, OWNP], BF16, kind="ExternalInput")
    idxE_in = nc.dram_tensor("idxE", [128, TOTSLOT // 16], dt.int16,
                             kind="ExternalInput")
    idxO_in = nc.dram_tensor("idxO", [128, TOTSLOT // 16], dt.int16,
                             kind="ExternalInput")
    wcat0_in = nc.dram_tensor("wcat0", [F_IN, C0], BF16, kind="ExternalInput")
    wr0_in = nc.dram_tensor("wr0", [F_IN, F_IN], BF16, kind="ExternalInput")
    wcat1_in = nc.dram_tensor("wcat1", [F_IN, C1], BF16, kind="ExternalInput")
    biasr1_in = nc.dram_tensor("bias_row1", [1, C1], BF16, kind="ExternalInput")
    bias0_in = nc.dram_tensor("bias0", [128, KCH], F32, kind="ExternalInput")
    out_own = nc.dram_tensor("out_own", [OWNP, OUT], F32, kind="ExternalOutput")

    tbl0_shard = nc.dram_tensor("tbl0_shard", [OWNP, ROW0], BF16)
    tbl1_shard = nc.dram_tensor("tbl1_shard", [OWNP, ROW1], BF16)
    reg0 = nc.dram_tensor("reg0", [VROWS, 2 * ROW0], BF16, addr_space="Shared")
    reg1 = nc.dram_tensor("reg1", [VROWS, 2 * ROW1], BF16, addr_space="Shared")
    rg = [list(range(C))]

    with tile.TileContext(nc) as tc, ExitStack() as ctx:
        const = ctx.enter_context(tc.tile_pool(name="const", bufs=1))
        wcat0_t = const.tile([128, KCH, C0], BF16)
        nc.sync.dma_start(wcat0_t[:], wcat0_in[:, :].rearrange(
            "(k p) c -> p k c", p=128))
        wr0_t = const.tile([128, KCH, F_IN], BF16)
        nc.sync.dma_start(wr0_t[:], wr0_in[:, :].rearrange(
            "(k p) c -> p k c", p=128))
        wcat1_t = const.tile([128, KCH, C1], BF16)
        nc.sync.dma_start(wcat1_t[:], wcat1_in[:, :].rearrange(
            "(k p) c -> p k c", p=128))
        biasr1_t = const.tile([1, C1], BF16)
        nc.sync.dma_start(biasr1_t[:], biasr1_in[:])
        bias0_t = const.tile([128, KCH], F32)
        nc.sync.dma_start(bias0_t[:], bias0_in[:])
        ones_t = const.tile([1, 128], BF16)
        nc.vector.memset(ones_t[:], 1.0)
        idxE_t = const.tile([128, TOTSLOT // 16], dt.int16)
        nc.sync.dma_start(idxE_t[:], idxE_in[:])
        idxO_t = const.tile([128, TOTSLOT // 16], dt.int16)
        nc.sync.dma_start(idxO_t[:], idxO_in[:])

        persist = ctx.enter_context(tc.tile_pool(name="persist", bufs=1))
        res0T = persist.tile([128, KCH, OWNP], BF16)
        h1T = persist.tile([128, KCH, OWNP], BF16)
        ad0 = persist.tile([128, NW, H], BF16)
        ad1 = persist.tile([128, NW, 1 + OUT], BF16)

        for _rep in range(repeat):
            # ---------------- setup: table0 rows, res0T, sentinels ----------
            with tc.tile_pool(name="xp", bufs=1) as xp, \
                 tc.tile_pool(name="sps", bufs=2, space="PSUM") as sps, \
                 tc.tile_pool(name="ssb", bufs=2) as ssb:
                xT_t = xp.tile([128, KCH, OWNP], BF16)
                nc.sync.dma_start(xT_t[:], xT_in[:, :].rearrange(
                    "(k p) n -> p k n", p=128))
                for w in range(NW):
                    ps = sps.tile([128, C0], F32, tag="t0")
                    for k in range(KCH):
                        nc.tensor.matmul(ps[:], xT_t[:, k, w * 128:(w + 1) * 128],
                                         wcat0_t[:, k, :],
                                         start=(k == 0), stop=(k == KCH - 1))
                    st = ssb.tile([128, C0], BF16, tag="st0")
                    nc.scalar.copy(st[:], ps[:])
                    nc.sync.dma_start(tbl0_shard[w * 128:(w + 1) * 128, :C0],
                                      st[:])
                # kick the collective off now so it overlaps the res0T
                # matmuls below (it only needs tbl0_shard)
                nc.sync.dma_start(
                    ad0[:], tbl0_shard[:, F_IN + H:F_IN + 2 * H].rearrange(
                        "(w p) e -> p w e", p=128))
                if NOCOLL:
                    nc.sync.dma_start(
                        reg0[2:2 + OWNP // 2, :],
                        tbl0_shard[:, :].rearrange("(v t) e -> v (t e)", t=2))
                else:
                    nc.gpsimd.collective_compute(
                        "AllGather", AL.bypass, replica_groups=rg,
                        ins=[tbl0_shard[:].opt()], outs=[reg0[2:, :].opt()])
                RT = 512
                for fc in range(KCH):
                    for n0 in range(0, OWNP, RT):
                        n1 = min(n0 + RT, OWNP)
                        ps = sps.tile([128, RT], F32, tag="r0")
                        for k in range(KCH):
                            nc.tensor.matmul(
                                ps[:, :n1 - n0],
                                wr0_t[:, k, fc * 128:(fc + 1) * 128],
                                xT_t[:, k, n0:n1],
                                start=(k == 0), stop=(k == KCH - 1))
                        nc.scalar.activation(res0T[:, fc, n0:n1],
                                             ps[:, :n1 - n0],
                                             ACT.Identity,
                                             bias=bias0_t[:, fc:fc + 1])
                # sentinels: view rows 0..1  (= table rows 0..3).
                # row 0 = zeros; row 1 starts with the PAD row (al = -1e30).
                sz = ssb.tile([1, 2 * ROW0], BF16, tag="sz")
                nc.vector.memset(sz[:], 0.0)
                sp = ssb.tile([1, 2 * ROW0], BF16, tag="sp")
                nc.vector.memset(sp[:], 0.0)
                nc.vector.memset(sp[:, F_IN:F_IN + H], -1e30)
                nc.sync.dma_start(reg0[0:1, :], sz[:])
                nc.sync.dma_start(reg0[1:2, :], sp[:])
                s1z = ssb.tile([1, 2 * ROW1], BF16, tag="s1z")
                nc.vector.memset(s1z[:], 0.0)
                s1p = ssb.tile([1, 2 * ROW1], BF16, tag="s1p")
                nc.vector.memset(s1p[:], 0.0)
                nc.vector.memset(s1p[:, OUT:OUT + 1], -1e30)
                nc.sync.dma_start(reg1[0:1, :], s1z[:])
                nc.sync.dma_start(reg1[1:2, :], s1p[:])

            # ---------------- layer 0 ----------------
            def gather_batch(gp, go, idx_off, slots, elem, regv_e, regv_o,
                             step):
                G = gp.tile([128, slots // 128, elem], BF16, tag="G")
                if NOGATHER:
                    nc.vector.memset(G[:], 0.0)
                    return G
                for c0 in range(0, slots, GCAP):
                    csl = min(GCAP, slots - c0)
                    Go = go.tile([128, GCAP // 128, elem], BF16, tag="Go")
                    nc.gpsimd.dma_gather(
                        out_ap=G[:, c0 // 128:(c0 + csl) // 128, :],
                        in_ap=regv_e,
                        idxs_ap=idxE_t[:, (idx_off + c0) // 16:
                                       (idx_off + c0 + csl) // 16],
                        num_idxs=csl, num_idxs_reg=csl, elem_size=elem,
                        elem_step=step)
                    nc.gpsimd.dma_gather(
                        out_ap=Go[:, :csl // 128, :],
                        in_ap=regv_o,
                        idxs_ap=idxO_t[:, (idx_off + c0) // 16:
                                       (idx_off + c0 + csl) // 16],
                        num_idxs=csl, num_idxs_reg=csl, elem_size=elem,
                        elem_step=step)
                    nc.vector.tensor_tensor(
                        G[:, c0 // 128:(c0 + csl) // 128, :],
                        G[:, c0 // 128:(c0 + csl) // 128, :],
                        Go[:, :csl // 128, :], AL.add)
                return G

            idx_off = 0
            with tc.tile_pool(name="g0", bufs=1) as gp, \
                 tc.tile_pool(name="go0", bufs=2) as go, \
                 tc.tile_pool(name="wk0", bufs=2) as wk:
                for (w0, nw_b, K) in batches:
                    slots = nw_b * K * 128
                    G = gather_batch(gp, go, idx_off, slots, ROW0,
                                     reg0[:, :ROW0], reg0[:, ROW0:],
                                     2 * ROW0)
                    idx_off += slots
                    Gv = G[:].rearrange("p (w k) e -> p w k e", w=nw_b)
                    s = wk.tile([128, nw_b, K, H], F32, tag="s")
                    nc.vector.tensor_tensor(
                        s[:], Gv[:, :, :, F_IN:F_IN + H],
                        ad0[:, w0:w0 + nw_b, :].unsqueeze(2)
                            .broadcast_to((128, nw_b, K, H)), AL.add)
                    nc.vector.scalar_tensor_tensor(s[:], s[:], NEG_SLOPE,
                                                   s[:], AL.mult, AL.max)
                    # per-lane max-shift: keeps exp args <= 0 (softmax
                    # shift-invariant; ACT exp is most accurate there)
                    mx = wk.tile([128, nw_b, H], F32, tag="mx")
                    nc.vector.reduce_max(
                        mx[:].unsqueeze(3),
                        s[:].rearrange("p w k h -> p w h k"),
                        axis=mybir.AxisListType.X)
                    nc.vector.tensor_tensor(
                        s[:], s[:],
                        mx[:].unsqueeze(2).broadcast_to((128, nw_b, K, H)),
                        AL.subtract)
                    Ex = wk.tile([128, nw_b, K, H], BF16, tag="E")
                    nc.scalar.activation(Ex[:], s[:], ACT.Exp)
                    nc.vector.tensor_tensor(
                        G[:, :, :F_IN].rearrange("p m (h d) -> p m h d", h=H),
                        G[:, :, :F_IN].rearrange("p m (h d) -> p m h d", h=H),
                        Ex[:].rearrange("p w k h -> p (w k) h").unsqueeze(3)
                            .broadcast_to((128, nw_b * K, H, D)), AL.mult)
                    nc.vector.tensor_copy(Gv[:, :, :, F_IN:F_IN + H], Ex[:])
                    U = wk.tile([128, nw_b, F_IN + H], F32, tag="U")
                    nc.vector.reduce_sum(
                        U[:].unsqueeze(3),
                        Gv[:, :, :, :F_IN + H].rearrange("p w k e -> p w e k"),
                        axis=mybir.AxisListType.X)
                    nc.vector.tensor_scalar(U[:, :, F_IN:], U[:, :, F_IN:],
                                            1e-16, None, AL.add)
                    rcp = wk.tile([128, nw_b, H], F32, tag="rcp")
                    nc.vector.reciprocal(rcp[:], U[:, :, F_IN:])
                    o0 = wk.tile([128, nw_b, F_IN], F32, tag="o0")
                    nc.vector.tensor_tensor(
                        o0[:].rearrange("p w (h d) -> p w h d", h=H),
                        U[:, :, :F_IN].rearrange("p w (h d) -> p w h d", h=H),
                        rcp[:].unsqueeze(3).broadcast_to((128, nw_b, H, D)),
                        AL.mult)
                    rT = wk.tile([128, nw_b, KCH, 128], BF16, tag="rT")
                    for wl in range(nw_b):
                        for k in range(KCH):
                            nc.sync.dma_start(
                                rT[:, wl, k, :],
                                res0T[:, k, (w0 + wl) * 128:(w0 + wl + 1) * 128],
                                transpose=True)
                    nc.vector.tensor_tensor(
                        o0[:], o0[:],
                        rT[:].rearrange("p w k e -> p w (k e)"), AL.add)
                    # ELU
                    mn = wk.tile([128, nw_b, F_IN], F32, tag="mn")
                    nc.vector.tensor_scalar(mn[:], o0[:], 0.0, None, AL.min)
                    nc.scalar.activation(mn[:], mn[:], ACT.Exp)
                    nc.vector.tensor_scalar(o0[:], o0[:], 0.0, None, AL.max)
                    nc.vector.tensor_tensor(o0[:], o0[:], mn[:], AL.add)
                    h1 = wk.tile([128, nw_b, F_IN], BF16, tag="h1")
                    nc.vector.tensor_scalar(h1[:], o0[:], 1.0, None,
                                            AL.subtract)
                    for wl in range(nw_b):
                        for k in range(KCH):
                            nc.sync.dma_start(
                                h1T[:, k, (w0 + wl) * 128:(w0 + wl + 1) * 128],
                                h1[:, wl, k * 128:(k + 1) * 128],
                                transpose=True)

            # ---------------- table1 rows ----------------
            with tc.tile_pool(name="t1ps", bufs=2, space="PSUM") as tps, \
                 tc.tile_pool(name="t1sb", bufs=2) as tsb:
                for w in range(NW):
                    ps = tps.tile([128, C1], F32, tag="t1")
                    for k in range(KCH):
                        nc.tensor.matmul(ps[:], h1T[:, k, w * 128:(w + 1) * 128],
                                         wcat1_t[:, k, :],
                                         start=(k == 0), stop=False)
                    nc.tensor.matmul(ps[:], ones_t[:], biasr1_t[:],
                                     start=False, stop=True)
                    st = tsb.tile([128, C1], BF16, tag="st1")
                    nc.scalar.copy(st[:], ps[:])
                    nc.sync.dma_start(tbl1_shard[w * 128:(w + 1) * 128, :C1],
                                      st[:])
            nc.sync.dma_start(
                ad1[:], tbl1_shard[:, OUT + 1:2 * OUT + 2].rearrange(
                    "(w p) e -> p w e", p=128))

            if NOCOLL:
                nc.sync.dma_start(
                    reg1[2:2 + OWNP // 2, :],
                    tbl1_shard[:, :].rearrange("(v t) e -> v (t e)", t=2))
            else:
                nc.gpsimd.collective_compute(
                    "AllGather", AL.bypass, replica_groups=rg,
                    ins=[tbl1_shard[:].opt()], outs=[reg1[2:, :].opt()])

            # ---------------- layer 1 ----------------
            GE1 = 128   # gather elems (covers hW1|als)
            idx_off = 0
            with tc.tile_pool(name="g1", bufs=1) as gp, \
                 tc.tile_pool(name="go1", bufs=2) as go, \
                 tc.tile_pool(name="wk1", bufs=2) as wk:
                for (w0, nw_b, K) in batches:
                    slots = nw_b * K * 128
                    G = gather_batch(gp, go, idx_off, slots, GE1,
                                     reg1[:, :GE1], reg1[:, ROW1:ROW1 + GE1],
                                     2 * ROW1)
                    idx_off += slots
                    Gv = G[:].rearrange("p (w k) e -> p w k e", w=nw_b)
                    s = wk.tile([128, nw_b, K, 1], F32, tag="s")
                    nc.vector.tensor_tensor(
                        s[:], Gv[:, :, :, OUT:OUT + 1],
                        ad1[:, w0:w0 + nw_b, 0:1].unsqueeze(2)
                            .broadcast_to((128, nw_b, K, 1)), AL.add)
                    nc.vector.scalar_tensor_tensor(s[:], s[:], NEG_SLOPE,
                                                   s[:], AL.mult, AL.max)
                    mx = wk.tile([128, nw_b, 1], F32, tag="mx")
                    nc.vector.reduce_max(
                        mx[:].unsqueeze(3),
                        s[:].rearrange("p w k h -> p w h k"),
                        axis=mybir.AxisListType.X)
                    nc.vector.tensor_tensor(
                        s[:], s[:],
                        mx[:].unsqueeze(2).broadcast_to((128, nw_b, K, 1)),
                        AL.subtract)
                    Ex = wk.tile([128, nw_b, K, 1], BF16, tag="E")
                    nc.scalar.activation(Ex[:], s[:], ACT.Exp)
                    nc.vector.tensor_tensor(
                        G[:, :, :OUT], G[:, :, :OUT],
                        Ex[:].rearrange("p w k h -> p (w k) h")
                            .broadcast_to((128, nw_b * K, OUT)), AL.mult)
                    nc.vector.tensor_copy(Gv[:, :, :, OUT:OUT + 1], Ex[:])
                    U = wk.tile([128, nw_b, OUT + 1], F32, tag="U")
                    nc.vector.reduce_sum(
                        U[:].unsqueeze(3),
                        Gv[:, :, :, :OUT + 1].rearrange("p w k e -> p w e k"),
                        axis=mybir.AxisListType.X)
                    nc.vector.tensor_scalar(U[:, :, OUT:], U[:, :, OUT:],
                                            1e-16, None, AL.add)
                    rcp = wk.tile([128, nw_b, 1], F32, tag="rcp")
                    nc.vector.reciprocal(rcp[:], U[:, :, OUT:])
                    o = wk.tile([128, nw_b, OUT], F32, tag="o")
                    nc.vector.tensor_tensor(
                        o[:], U[:, :, :OUT],
                        rcp[:].broadcast_to((128, nw_b, OUT)), AL.mult)
                    nc.vector.tensor_tensor(o[:], o[:],
                                            ad1[:, w0:w0 + nw_b, 1:],
                                            AL.add)
                    # log_softmax with max-shift
                    mxo = wk.tile([128, nw_b, 1], F32, tag="mxo")
                    nc.vector.reduce_max(mxo[:], o[:], axis=mybir.AxisListType.X)
                    nc.vector.tensor_tensor(
                        o[:], o[:], mxo[:].broadcast_to((128, nw_b, OUT)),
                        AL.subtract)
                    ev = wk.tile([128, nw_b, OUT], F32, tag="ev")
                    nc.scalar.activation(ev[:], o[:], ACT.Exp)
                    sv = wk.tile([128, nw_b, 1], F32, tag="sv")
                    nc.vector.reduce_sum(sv[:], ev[:], axis=mybir.AxisListType.X)
                    nc.scalar.activation(sv[:], sv[:], ACT.Ln)
                    nc.vector.tensor_tensor(
                        o[:], o[:], sv[:].broadcast_to((128, nw_b, OUT)),
                        AL.subtract)
                    nc.sync.dma_start(
                        out_own[w0 * 128:(w0 + nw_b) * 128, :].rearrange(
                            "(w p) e -> p w e", p=128), o[:])

    nc.compile()
    return nc


# ----------------------------------------------------------------- entrypoint

def kernel(x, edge_index, W0, a_src0, a_dst0, b0, Wr0, br0,
           W1, a_src1, a_dst1, b1, Wr1, br1):
    """Full-input GAT kernel: shards across 8 NeuronCores internally."""
    x = np.asarray(x)
    edge_index = np.asarray(edge_index)
    N, F_in = x.shape
    E = edge_index.shape[1]
    H, D = np.asarray(a_src0).shape
    OUT = np.asarray(a_src1).shape[1]
    cfg = make_cfg(N, E, F_in, H, D, OUT, _CORES)
    weights = dict(
        W0=np.asarray(W0, np.float32), a_src0=np.asarray(a_src0, np.float32),
        a_dst0=np.asarray(a_dst0, np.float32), b0=np.asarray(b0, np.float32),
        Wr0=np.asarray(Wr0, np.float32), br0=np.asarray(br0, np.float32),
        W1=np.asarray(W1, np.float32), a_src1=np.asarray(a_src1, np.float32),
        a_dst1=np.asarray(a_dst1, np.float32), b1=np.asarray(b1, np.float32),
        Wr1=np.asarray(Wr1, np.float32), br1=np.asarray(br1, np.float32))
    in_maps, meta = prepare(x.astype(np.float32), edge_index, weights, cfg)
    nc = build(cfg, meta)
    from concourse.bass_utils import run_bass_kernel_spmd
    res = run_bass_kernel_spmd(nc, in_maps, list(range(_CORES)))
    core_of, pos_of = meta["core_of"], meta["pos_of"]
    per_core = [np.asarray(res.results[c]["out_own"], np.float32)
                for c in range(_CORES)]
    stacked = np.stack(per_core)                       # [C, OWNP, OUT]
    out = stacked[core_of, pos_of]                     # [N, OUT]
    return out
